# revision 3
# baseline (speedup 1.0000x reference)
"""Ragged-sequence attention pooling on 8 TRN2 NeuronCores.

reference:
    scores[b,t] = sum_d seq[b,t,d] * cond[b,d]
    scores masked with -1e20 where t >= lens[b]
    out[b,:]   = softmax_t(scores) @ seq[b]   -> [B, D]

Sharding: data-parallel over B (32 batches -> 4 per core). Each core
streams its 64 MiB of seq exactly once (memory-bound target).

Per-core algorithm (per batch, T=4096 = 2 halves of 16 tiles of 128 t):
  - DMA slabs [128, 4, 1024] (t on partitions, 4 KiB contiguous/partition)
  - DVE tensor_tensor_reduce: prod = tile * cond_bcast,
        score[:, i] = reduce_add(prod) + maskbias(tile i)   (fused, 1 op/tile)
  - per half: rowmax -> PE transpose -> max -> PE broadcast -> ScalarE
        exp(score - m_h) with fused row-sum; 32 accumulating PE matmuls
        p_exp^T @ seq -> acc_h [1, 1024] in PSUM; l_h = sum p_exp
  - per batch: merge halves with weights w_h = exp(m_h - M),
        out = (w0*acc0 + w1*acc1) / (w0*l0 + w1*l1)
"""

import numpy as np

import concourse.bacc as bacc
import concourse.bass as bass
import concourse.tile as tile
from concourse import mybir
from concourse.bass_utils import run_bass_kernel_spmd

F32 = mybir.dt.float32
ALU = mybir.AluOpType
AF = mybir.ActivationFunctionType

B, T, D = 32, 4096, 1024
NCORES = 8
BPC = B // NCORES          # batches per core = 4
P = 128                    # partitions / timesteps per tile
NT = T // P                # 32 tiles per batch
NHALF = 2                  # softmax blocks per batch
HT = NT // NHALF           # 16 tiles per half
SLAB = 4                   # tiles per DMA
NEG_INF = -1e20


def build_program():
    nc = bacc.Bacc("TRN2", target_bir_lowering=False, debug=False,
                   num_devices=NCORES)

    seq = nc.dram_tensor("seq", [BPC, T, D], F32, kind="ExternalInput")
    cond = nc.dram_tensor("cond", [BPC, D], F32, kind="ExternalInput")
    maskb = nc.dram_tensor("maskb", [P, BPC, NT], F32, kind="ExternalInput")
    ident = nc.dram_tensor("ident", [P, P], F32, kind="ExternalInput")
    out = nc.dram_tensor("out", [BPC, D], F32, kind="ExternalOutput")

    with tile.TileContext(nc) as tc:
        with (
            tc.tile_pool(name="singles", bufs=1) as singles,
            tc.tile_pool(name="seqp", bufs=7) as seqp,
            tc.tile_pool(name="prodp", bufs=3) as prodp,
            tc.tile_pool(name="scorep", bufs=2) as scorep,
            tc.tile_pool(name="pexpp", bufs=2) as pexpp,
            tc.tile_pool(name="statp", bufs=3) as statp,
            tc.tile_pool(name="batchp", bufs=2) as batchp,
            tc.tile_pool(name="cstage", bufs=2) as cstage,
            tc.tile_pool(name="accp", bufs=4, space="PSUM") as accp,
            tc.tile_pool(name="miscp", bufs=2, space="PSUM") as miscp,
        ):
            # constants
            ident_sb = singles.tile([P, P], F32)
            nc.sync.dma_start(out=ident_sb, in_=ident[:])
            mask_sb = singles.tile([P, BPC, NT], F32)
            nc.sync.dma_start(out=mask_sb, in_=maskb[:])
            # cond broadcast to all 128 partitions: [128, BPC, D]
            cond_sb = singles.tile([P, BPC, D], F32)
            cond_ap = cond[:]
            cond_bcast = bass.AP(
                tensor=cond_ap.tensor,
                offset=cond_ap.offset,
                ap=[[0, P]] + [list(x) for x in cond_ap.ap],
            )
            nc.sync.dma_start(out=cond_sb, in_=cond_bcast)
            ones_col = singles.tile([P, 1], F32)
            nc.vector.memset(ones_col, 1.0)
            ones_row = singles.tile([1, P], F32)
            nc.vector.memset(ones_row, 1.0)

            for b in range(BPC):
                m2 = batchp.tile([1, NHALF], F32, tag="m2")
                l2 = batchp.tile([1, NHALF], F32, tag="l2")
                haccs = []
                for h in range(NHALF):
                    scores = scorep.tile([P, HT], F32, tag="scores")
                    slabs = []
                    for s in range(HT // SLAB):
                        slab = seqp.tile([P, SLAB, D], F32, tag="slab")
                        r0 = h * (HT * P) + s * (SLAB * P)
                        src = seq[b, r0:r0 + SLAB * P, :].rearrange(
                            "(j p) d -> p j d", p=P)
                        nc.sync.dma_start(out=slab, in_=src)
                        slabs.append(slab)
                        for j in range(SLAB):
                            i = s * SLAB + j          # tile idx within half
                            prod = prodp.tile([P, D], F32, tag="prod")
                            # out = (slab * 1.0) * cond ; accum = row sum
                            nc.vector.scalar_tensor_tensor(
                                out=prod,
                                in0=slab[:, j, :],
                                scalar=1.0,
                                in1=cond_sb[:, b, :],
                                op0=ALU.mult,
                                op1=ALU.mult,
                                accum_out=scores[:, i:i + 1],
                            )

                    # apply length mask (additive -1e20 bias), then stats
                    nc.vector.tensor_add(
                        scores, scores, mask_sb[:, b, h * HT:(h + 1) * HT])
                    rowmax = statp.tile([P, 1], F32, tag="rowmax")
                    nc.vector.tensor_reduce(
                        out=rowmax, in_=scores, axis=mybir.AxisListType.X,
                        op=ALU.max)
                    tmax = miscp.tile([1, P], F32, tag="misc")
                    nc.tensor.transpose(tmax, rowmax, ident_sb)
                    nc.vector.tensor_reduce(
                        out=m2[:, h:h + 1], in_=tmax,
                        axis=mybir.AxisListType.X, op=ALU.max)
                    # broadcast m_h to all partitions (ones_row.T @ m)
                    mb = miscp.tile([P, 1], F32, tag="misc")
                    nc.tensor.matmul(mb, lhsT=ones_row, rhs=m2[:, h:h + 1],
                                     start=True, stop=True)
                    negm = statp.tile([P, 1], F32, tag="negm")
                    nc.scalar.activation(negm, mb, AF.Copy, scale=-1.0)
                    # p_exp = exp(scores - m_h), rowsum fused
                    pexp = pexpp.tile([P, HT], F32, tag="pexp")
                    rowsum = statp.tile([P, 1], F32, tag="rowsum")
                    nc.scalar.activation(pexp, scores, AF.Exp, bias=negm,
                                         scale=1.0, accum_out=rowsum)
                    # l_h = sum_p rowsum
                    lps = miscp.tile([1, 1], F32, tag="misc")
                    nc.tensor.matmul(lps, lhsT=rowsum, rhs=ones_col,
                                     start=True, stop=True)
                    nc.vector.tensor_copy(l2[:, h:h + 1], lps)

                    # weighted sum: acc[1, D] += pexp[:,i]^T @ seq_tile
                    accA = accp.tile([1, 512], F32, tag="acc")
                    accB = accp.tile([1, 512], F32, tag="acc")
                    for s in range(HT // SLAB):
                        slab = slabs[s]
                        for j in range(SLAB):
                            i = s * SLAB + j
                            st, sp = (i == 0), (i == HT - 1)
                            nc.tensor.matmul(
                                accA, lhsT=pexp[:, i:i + 1],
                                rhs=slab[:, j, 0:512], start=st, stop=sp)
                            nc.tensor.matmul(
                                accB, lhsT=pexp[:, i:i + 1],
                                rhs=slab[:, j, 512:1024], start=st, stop=sp)
                    haccs.append((accA, accB))

                # merge halves: w_h = exp(m_h - M)
                Mx = batchp.tile([1, 1], F32, tag="Mx")
                nc.vector.tensor_reduce(out=Mx, in_=m2,
                                        axis=mybir.AxisListType.X, op=ALU.max)
                nMx = batchp.tile([1, 1], F32, tag="nMx")
                nc.vector.tensor_scalar_mul(nMx, Mx, -1.0)
                w = batchp.tile([1, NHALF], F32, tag="w")
                nc.scalar.activation(w, m2, AF.Exp, bias=nMx, scale=1.0)
                dscr = batchp.tile([1, NHALF], F32, tag="dscr")
                den = batchp.tile([1, 1], F32, tag="den")
                nc.vector.scalar_tensor_tensor(
                    out=dscr, in0=w, scalar=1.0, in1=l2,
                    op0=ALU.mult, op1=ALU.mult, accum_out=den)
                rden = batchp.tile([1, 1], F32, tag="rden")
                nc.vector.reciprocal(rden, den)

                (a0, b0), (a1, b1) = haccs
                s0 = cstage.tile([1, D], F32, tag="s0")
                nc.scalar.activation(s0[:, 0:512], a0, AF.Copy,
                                     scale=w[:, 0:1])
                nc.scalar.activation(s0[:, 512:1024], b0, AF.Copy,
                                     scale=w[:, 0:1])
                fin = cstage.tile([1, D], F32, tag="fin")
                nc.vector.scalar_tensor_tensor(
                    out=fin[:, 0:512], in0=a1, scalar=w[:, 1:2],
                    in1=s0[:, 0:512], op0=ALU.mult, op1=ALU.add)
                nc.vector.scalar_tensor_tensor(
                    out=fin[:, 512:1024], in0=b1, scalar=w[:, 1:2],
                    in1=s0[:, 512:1024], op0=ALU.mult, op1=ALU.add)
                ostage = cstage.tile([1, D], F32, tag="ostage")
                nc.scalar.activation(ostage, fin, AF.Copy, scale=rden)
                nc.sync.dma_start(out=out[b:b + 1, :], in_=ostage)

    nc.compile()
    return nc


_NC_CACHE = None


def _get_program():
    global _NC_CACHE
    if _NC_CACHE is None:
        _NC_CACHE = build_program()
    return _NC_CACHE


def make_in_maps(seq, lens, cond):
    ident = np.eye(P, dtype=np.float32)
    t_of = (np.arange(NT)[None, :] * P + np.arange(P)[:, None])  # [P, NT]
    in_maps = []
    for c in range(NCORES):
        bs = slice(c * BPC, (c + 1) * BPC)
        lens_c = np.asarray(lens[bs]).astype(np.int64)  # [BPC]
        # maskb[p, b, i] = 0 if (i*128+p) < len else NEG_INF
        mb = np.where(t_of[:, None, :] < lens_c[None, :, None],
                      0.0, NEG_INF).astype(np.float32)
        in_maps.append({
            "seq": np.ascontiguousarray(seq[bs]).astype(np.float32),
            "cond": np.ascontiguousarray(cond[bs]).astype(np.float32),
            "maskb": np.ascontiguousarray(mb),
            "ident": ident,
        })
    return in_maps


def run(seq, lens, cond, trace=False, **kw):
    nc = _get_program()
    in_maps = make_in_maps(seq, lens, cond)
    res = run_bass_kernel_spmd(nc, in_maps, core_ids=list(range(NCORES)),
                               trace=trace, **kw)
    outs = np.concatenate([res.results[i]["out"] for i in range(NCORES)],
                          axis=0)
    return outs, res


def kernel(seq, lens, cond):
    outs, _ = run(seq, lens, cond)
    return outs


# revision 7
# speedup vs baseline: 1.1668x; 1.1668x over previous
"""Ragged-sequence attention pooling on 8 TRN2 NeuronCores.

reference:
    scores[b,t] = sum_d seq[b,t,d] * cond[b,d]
    scores masked with -1e20 where t >= lens[b]
    out[b,:]   = softmax_t(scores) @ seq[b]   -> [B, D]

Sharding: data-parallel over B (32 batches -> 4 per core). Each core
streams its 64 MiB of seq exactly once (memory-bound target).

Per-core algorithm (per batch, T=4096 = 2 halves of 16 tiles of 128 t):
  - DMA slabs [128, 4, 1024] (t on partitions, 4 KiB contiguous/partition)
  - DVE tensor_tensor_reduce: prod = tile * cond_bcast,
        score[:, i] = reduce_add(prod) + maskbias(tile i)   (fused, 1 op/tile)
  - per half: rowmax -> PE transpose -> max -> PE broadcast -> ScalarE
        exp(score - m_h) with fused row-sum; 32 accumulating PE matmuls
        p_exp^T @ seq -> acc_h [1, 1024] in PSUM; l_h = sum p_exp
  - per batch: merge halves with weights w_h = exp(m_h - M),
        out = (w0*acc0 + w1*acc1) / (w0*l0 + w1*l1)
"""

import numpy as np

import concourse.bacc as bacc
import concourse.bass as bass
import concourse.tile as tile
from concourse import mybir
from concourse.bass_utils import run_bass_kernel_spmd

F32 = mybir.dt.float32
BF16 = mybir.dt.bfloat16
ALU = mybir.AluOpType
AF = mybir.ActivationFunctionType

B, T, D = 32, 4096, 1024
NCORES = 8
BPC = B // NCORES          # batches per core = 4
P = 128                    # partitions / timesteps per tile
NT = T // P                # 32 tiles per batch
NHALF = 2                  # softmax blocks per batch
HT = NT // NHALF           # 16 tiles per half
SLAB = 4                   # tiles per DMA
NEG_INF = -1e20


def build_program():
    nc = bacc.Bacc("TRN2", target_bir_lowering=False, debug=False,
                   num_devices=NCORES)

    seq = nc.dram_tensor("seq", [BPC, T, D], F32, kind="ExternalInput")
    cond = nc.dram_tensor("cond", [BPC, D], F32, kind="ExternalInput")
    maskb = nc.dram_tensor("maskb", [P, BPC, NT], F32, kind="ExternalInput")
    ident = nc.dram_tensor("ident", [P, P], F32, kind="ExternalInput")
    out = nc.dram_tensor("out", [BPC, D], F32, kind="ExternalOutput")

    with tile.TileContext(nc) as tc:
        with (
            tc.tile_pool(name="singles", bufs=1) as singles,
            tc.tile_pool(name="seqp", bufs=4) as seqp,
            tc.tile_pool(name="bfp", bufs=6) as bfp,
            tc.tile_pool(name="prodp", bufs=3) as prodp,
            tc.tile_pool(name="scorep", bufs=2) as scorep,
            tc.tile_pool(name="pexpp", bufs=2) as pexpp,
            tc.tile_pool(name="statp", bufs=3) as statp,
            tc.tile_pool(name="batchp", bufs=2) as batchp,
            tc.tile_pool(name="cstage", bufs=2) as cstage,
            tc.tile_pool(name="accp", bufs=4, space="PSUM") as accp,
            tc.tile_pool(name="miscp", bufs=2, space="PSUM") as miscp,
        ):
            # constants
            ident_sb = singles.tile([P, P], F32)
            nc.sync.dma_start(out=ident_sb, in_=ident[:])
            mask_sb = singles.tile([P, BPC, NT], F32)
            nc.sync.dma_start(out=mask_sb, in_=maskb[:])
            # cond broadcast to all 128 partitions: [128, BPC, D]
            cond_sb = singles.tile([P, BPC, D], F32)
            cond_ap = cond[:]
            cond_bcast = bass.AP(
                tensor=cond_ap.tensor,
                offset=cond_ap.offset,
                ap=[[0, P]] + [list(x) for x in cond_ap.ap],
            )
            nc.sync.dma_start(out=cond_sb, in_=cond_bcast)
            ones_col = singles.tile([P, 1], F32)
            nc.vector.memset(ones_col, 1.0)
            ones_row = singles.tile([1, P], F32)
            nc.vector.memset(ones_row, 1.0)

            for b in range(BPC):
                m2 = batchp.tile([1, NHALF], F32, tag="m2")
                l2 = batchp.tile([1, NHALF], F32, tag="l2")
                haccs = []
                for h in range(NHALF):
                    scores = scorep.tile([P, HT], F32, tag="scores")
                    slabs = []
                    for s in range(HT // SLAB):
                        slab = seqp.tile([P, SLAB, D], F32, tag="slab")
                        r0 = h * (HT * P) + s * (SLAB * P)
                        src = seq[b, r0:r0 + SLAB * P, :].rearrange(
                            "(j p) d -> p j d", p=P)
                        nc.sync.dma_start(out=slab, in_=src)
                        # bf16 copy for the weighted-sum matmuls (PE
                        # streams bf16 4x faster than f32)
                        slab_bf = bfp.tile([P, SLAB, D], BF16, tag="slabbf")
                        nc.scalar.activation(slab_bf, slab, AF.Copy,
                                             scale=1.0)
                        slabs.append(slab_bf)
                        for j in range(SLAB):
                            i = s * SLAB + j          # tile idx within half
                            prod = prodp.tile([P, D], F32, tag="prod")
                            # out = (slab * 1.0) * cond ; accum = row sum
                            nc.vector.scalar_tensor_tensor(
                                out=prod,
                                in0=slab[:, j, :],
                                scalar=1.0,
                                in1=cond_sb[:, b, :],
                                op0=ALU.mult,
                                op1=ALU.mult,
                                accum_out=scores[:, i:i + 1],
                            )

                    # apply length mask (additive -1e20 bias), then stats
                    nc.vector.tensor_add(
                        scores, scores, mask_sb[:, b, h * HT:(h + 1) * HT])
                    rowmax = statp.tile([P, 1], F32, tag="rowmax")
                    nc.vector.tensor_reduce(
                        out=rowmax, in_=scores, axis=mybir.AxisListType.X,
                        op=ALU.max)
                    tmax = miscp.tile([1, P], F32, tag="misc")
                    nc.tensor.transpose(tmax, rowmax, ident_sb)
                    nc.vector.tensor_reduce(
                        out=m2[:, h:h + 1], in_=tmax,
                        axis=mybir.AxisListType.X, op=ALU.max)
                    # broadcast m_h to all partitions (ones_row.T @ m)
                    mb = miscp.tile([P, 1], F32, tag="misc")
                    nc.tensor.matmul(mb, lhsT=ones_row, rhs=m2[:, h:h + 1],
                                     start=True, stop=True)
                    negm = statp.tile([P, 1], F32, tag="negm")
                    nc.scalar.activation(negm, mb, AF.Copy, scale=-1.0)
                    # p_exp = exp(scores - m_h), rowsum fused
                    pexp = pexpp.tile([P, HT], BF16, tag="pexp")
                    rowsum = statp.tile([P, 1], F32, tag="rowsum")
                    nc.scalar.activation(pexp, scores, AF.Exp, bias=negm,
                                         scale=1.0, accum_out=rowsum)
                    # l_h = sum_p rowsum
                    lps = miscp.tile([1, 1], F32, tag="misc")
                    nc.tensor.matmul(lps, lhsT=rowsum, rhs=ones_col,
                                     start=True, stop=True)
                    nc.vector.tensor_copy(l2[:, h:h + 1], lps)

                    # weighted sum: acc[1, D] += pexp[:,i]^T @ seq_tile
                    accA = accp.tile([1, 512], F32, tag="acc")
                    accB = accp.tile([1, 512], F32, tag="acc")
                    for s in range(HT // SLAB):
                        slab = slabs[s]
                        for j in range(SLAB):
                            i = s * SLAB + j
                            st, sp = (i == 0), (i == HT - 1)
                            nc.tensor.matmul(
                                accA, lhsT=pexp[:, i:i + 1],
                                rhs=slab[:, j, 0:512], start=st, stop=sp)
                            nc.tensor.matmul(
                                accB, lhsT=pexp[:, i:i + 1],
                                rhs=slab[:, j, 512:1024], start=st, stop=sp)
                    haccs.append((accA, accB))

                # merge halves: w_h = exp(m_h - M)
                Mx = batchp.tile([1, 1], F32, tag="Mx")
                nc.vector.tensor_reduce(out=Mx, in_=m2,
                                        axis=mybir.AxisListType.X, op=ALU.max)
                nMx = batchp.tile([1, 1], F32, tag="nMx")
                nc.vector.tensor_scalar_mul(nMx, Mx, -1.0)
                w = batchp.tile([1, NHALF], F32, tag="w")
                nc.scalar.activation(w, m2, AF.Exp, bias=nMx, scale=1.0)
                dscr = batchp.tile([1, NHALF], F32, tag="dscr")
                den = batchp.tile([1, 1], F32, tag="den")
                nc.vector.scalar_tensor_tensor(
                    out=dscr, in0=w, scalar=1.0, in1=l2,
                    op0=ALU.mult, op1=ALU.mult, accum_out=den)
                rden = batchp.tile([1, 1], F32, tag="rden")
                nc.vector.reciprocal(rden, den)

                (a0, b0), (a1, b1) = haccs
                s0 = cstage.tile([1, D], F32, tag="s0")
                nc.scalar.activation(s0[:, 0:512], a0, AF.Copy,
                                     scale=w[:, 0:1])
                nc.scalar.activation(s0[:, 512:1024], b0, AF.Copy,
                                     scale=w[:, 0:1])
                fin = cstage.tile([1, D], F32, tag="fin")
                nc.vector.scalar_tensor_tensor(
                    out=fin[:, 0:512], in0=a1, scalar=w[:, 1:2],
                    in1=s0[:, 0:512], op0=ALU.mult, op1=ALU.add)
                nc.vector.scalar_tensor_tensor(
                    out=fin[:, 512:1024], in0=b1, scalar=w[:, 1:2],
                    in1=s0[:, 512:1024], op0=ALU.mult, op1=ALU.add)
                ostage = cstage.tile([1, D], F32, tag="ostage")
                nc.scalar.activation(ostage, fin, AF.Copy, scale=rden)
                nc.sync.dma_start(out=out[b:b + 1, :], in_=ostage)

    nc.compile()
    return nc


_NC_CACHE = None


def _get_program():
    global _NC_CACHE
    if _NC_CACHE is None:
        _NC_CACHE = build_program()
    return _NC_CACHE


def make_in_maps(seq, lens, cond):
    ident = np.eye(P, dtype=np.float32)
    t_of = (np.arange(NT)[None, :] * P + np.arange(P)[:, None])  # [P, NT]
    in_maps = []
    for c in range(NCORES):
        bs = slice(c * BPC, (c + 1) * BPC)
        lens_c = np.asarray(lens[bs]).astype(np.int64)  # [BPC]
        # maskb[p, b, i] = 0 if (i*128+p) < len else NEG_INF
        mb = np.where(t_of[:, None, :] < lens_c[None, :, None],
                      0.0, NEG_INF).astype(np.float32)
        in_maps.append({
            "seq": np.ascontiguousarray(seq[bs]).astype(np.float32),
            "cond": np.ascontiguousarray(cond[bs]).astype(np.float32),
            "maskb": np.ascontiguousarray(mb),
            "ident": ident,
        })
    return in_maps


def run(seq, lens, cond, trace=False, **kw):
    nc = _get_program()
    in_maps = make_in_maps(seq, lens, cond)
    res = run_bass_kernel_spmd(nc, in_maps, core_ids=list(range(NCORES)),
                               trace=trace, **kw)
    outs = np.concatenate([res.results[i]["out"] for i in range(NCORES)],
                          axis=0)
    return outs, res


def kernel(seq, lens, cond):
    outs, _ = run(seq, lens, cond)
    return outs
